# revision 29
# baseline (speedup 1.0000x reference)
"""Trainium2 Bass kernel for NSHE-style GNN message passing.

  enc = relu(concat(feat_a@W_a+b_a, feat_b@W_b+b_b, feat_c@W_c+b_c))
  support = enc @ gcn_W + gcn_b
  msg = support[edge_src] * edge_weight[:, None]
  com = segment_sum(msg, edge_dst, N);  out = l2_normalize(com, axis=1)

Distribution (8 NeuronCores, one shared SPMD NEFF):
  - nodes are permuted into 8 balanced per-core slices (each slice mixes the
    three feature types so per-core phase-1 work is equal); core k computes
    `support` rows for its slice (bf16, rows padded to 256B for the gather
    granule), then an AllGather replicates the full N x 128 table.
  - edges are partitioned by destination slice; each core's edge stream is
    sorted by (psum-group, src-window, pair-window, dst) and padded so tile
    counts are identical on every core (one program serves all cores).
  - support[src] rows are fetched with dma_gather (SWDGE, int16 indices into
    25000-row windows, <=1024 idxs/instruction, spread over 4 SWDGE queues).
  - segment-sum runs on the tensor engine: per 128-edge tile a selection
    matrix M[e, j] = (j == dst_rel[e]) * w_e (bf16) is built with one DVE
    tensor_scalar; ONE matmul per tile computes psum[f, dst] += gt^T @ M
    with the gathered rows as the 64-col stationary operand and M as the
    256-col moving operand. Each PSUM bank holds two 256-dst pairs
    ([64, 512]); 12 pairs per group, two banks serve the PE transposes
    that stream each finished group back to row-major [dst, 64].
  - the [64, dst] accumulator is PE-transposed back to row-major [dst, 64],
    l2-normalized in one batched pass; the host undoes the permutation.
"""

import numpy as np

N_A, N_B, N_C = 100000, 60000, 40000
D = 64
D_IN = (512, 256, 128)
NCORES = 8

P = 128                  # partitions / edge-tile size
PAIR = 2 * P             # dst span covered by one tile's matmul
PAIRS_PER_GROUP = 12     # 6 psum banks x 2 pairs; 2 banks for transposes
GATHER_CHUNK = 1024      # idxs per dma_gather (SWDGE ring limit)
NQ = 4                   # SWDGE queues


class _Plan:
    pass


def _bf16(x):
    import ml_dtypes
    return np.asarray(x).astype(ml_dtypes.bfloat16)


def _make_plan(edge_src, edge_dst, edge_weight):
    """Host-side sharding: node permutation, uniform per-core edge schedule,
    operand arrays. Index manipulation only -- all float math runs on device
    (edge weights are moved, never combined, here)."""
    pl = _Plan()
    N = N_A + N_B + N_C
    SLICE = N // NCORES
    a_s, b_s, c_s = N_A // NCORES, N_B // NCORES, N_C // NCORES

    node_to_table = np.empty(N, dtype=np.int64)
    karr = np.arange(NCORES)
    for cnt, node0, off in ((a_s, 0, 0), (b_s, N_A, a_s), (c_s, N_A + N_B, a_s + b_s)):
        idx = node0 + (karr[:, None] * cnt + np.arange(cnt)[None, :])
        rows = SLICE * karr[:, None] + off + np.arange(cnt)[None, :]
        node_to_table[idx.ravel()] = rows.ravel()
    table_to_node = np.empty(N, dtype=np.int64)
    table_to_node[node_to_table] = np.arange(N)
    pl.N, pl.SLICE = N, SLICE
    pl.a_s, pl.b_s, pl.c_s = a_s, b_s, c_s
    pl.node_to_table, pl.table_to_node = node_to_table, table_to_node

    # src-side rows follow the split-AllGather layout: half h of every
    # core's slice is gathered into table_h[h] with rows [core, half-slice],
    # so src windows of 12500 rows each map to one core's half.
    HALF = SLICE // 2
    c_of = node_to_table // SLICE
    r_of = node_to_table % SLICE
    ag_row = (r_of // HALF) * (NCORES * HALF) + c_of * HALF + (r_of % HALF)
    pl.HALF = HALF
    src_t = ag_row[edge_src]
    dst_t = node_to_table[edge_dst]
    w = edge_weight

    core = dst_t // SLICE
    dst_local = dst_t - core * SLICE
    pair = dst_local // PAIR
    NPAIR = (SLICE + PAIR - 1) // PAIR
    NG = (NPAIR + PAIRS_PER_GROUP - 1) // PAIRS_PER_GROUP
    pl.NPAIR, pl.NG = NPAIR, NG
    pl.pairs_in_group = [min(PAIRS_PER_GROUP, NPAIR - g * PAIRS_PER_GROUP)
                         for g in range(NG)]
    g_of = pair // PAIRS_PER_GROUP
    sp_of = pair % PAIRS_PER_GROUP
    SW = SLICE                   # src window: 25000 contiguous table rows
    assert SW <= 32767           # = two cores' half-slices (int16-safe)
    NSW = N // SW
    pl.NSW, pl.SW = NSW, SW
    sw = src_t // SW

    cell = ((core * NG + g_of) * NSW + sw) * PAIRS_PER_GROUP + sp_of
    counts = np.bincount(cell, minlength=NCORES * NG * NSW * PAIRS_PER_GROUP)
    counts = counts.reshape(NCORES, NG, NSW, PAIRS_PER_GROUP)
    T = np.maximum(1, -(-counts.max(axis=0) // P))          # [NG, NSW, SPG]
    for g in range(NG):
        T[g, :, pl.pairs_in_group[g]:] = 0
    pl.T = T
    T_tot = int(T.sum())
    S_tot = T_tot * P
    pl.T_tot, pl.S_tot = T_tot, S_tot

    cell_sizes = (T * P).ravel()
    cb = np.zeros(len(cell_sizes) + 1, dtype=np.int64)
    np.cumsum(cell_sizes, out=cb[1:])
    pl.cell_base = cb[:-1].reshape(NG, NSW, PAIRS_PER_GROUP)

    gidx = np.zeros((NCORES, S_tot), dtype=np.int16)
    dstrel = np.full((NCORES, S_tot), -999.0, dtype=np.float32)
    wstream = np.zeros((NCORES, S_tot), dtype=np.float32)

    order = np.lexsort((src_t, sp_of, sw, g_of, core))
    srt_core = core[order]
    srt_cic = (g_of[order] * NSW + sw[order]) * PAIRS_PER_GROUP + sp_of[order]
    srt_src_rel = (src_t[order] - sw[order] * SW).astype(np.int16)
    srt_dst_rel = (dst_local[order] - pair[order] * PAIR).astype(np.float32)
    srt_w = w[order].astype(np.float32)

    flat_base = pl.cell_base.ravel()
    ncell_pc = NG * NSW * PAIRS_PER_GROUP
    for c in range(NCORES):
        m = srt_core == c
        cic = srt_cic[m]
        oc = np.bincount(cic, minlength=ncell_pc)
        within = (np.arange(len(cic)) - np.repeat(np.concatenate([[0], np.cumsum(oc)[:-1]]), oc))
        pos = flat_base[cic] + within
        gidx[c, pos] = srt_src_rel[m]
        dstrel[c, pos] = srt_dst_rel[m]
        wstream[c, pos] = srt_w[m]

    pl.dstloc = dstrel.reshape(NCORES, T_tot, P).transpose(0, 2, 1).copy()
    pl.wcol = wstream.reshape(NCORES, T_tot, P).transpose(0, 2, 1).copy()
    wrapped = gidx.reshape(NCORES, S_tot // 16, 16).transpose(0, 2, 1)
    pl.gidx_wrapped = np.ascontiguousarray(np.tile(wrapped, (1, 8, 1)))
    return pl


def _build_nc(pl, variant="full"):
    # variant: ablation knob for benchmarking ("full" for real use).
    # cumulative ladder: each step also drops downstream consumers so no
    # tile is read-but-never-written (the allocator rejects that).
    #   nomm: drop segment matmuls + flush + transpose/normalize content
    #   nomb: also drop M-build     nog: also drop gathers
    #   p1ag: skip whole phase-2 loop    nop1 / noag: drop that phase
    import concourse.bacc as bacc
    import concourse.mybir as mybir
    import concourse.tile as tile
    from concourse.bass import AP

    do_p1 = variant != "nop1"
    do_ag = variant != "noag"
    do_mm = variant not in ("nomm", "nomb", "nog")
    do_mb = variant not in ("nomb", "nog")
    do_gather = variant != "nog"
    do_p2 = variant != "p1ag"

    dt = mybir.dt
    Alu = mybir.AluOpType
    Act = mybir.ActivationFunctionType
    NG, NSW, SLICE, T = pl.NG, pl.NSW, pl.SLICE, pl.T
    T_tot, S_tot, N = pl.T_tot, pl.S_tot, pl.N
    NBLK = (pl.NPAIR * PAIR) // P           # 196 transposed output blocks
    OUT_W = NBLK * D                        # 12544
    ACC_W = pl.NPAIR * PAIR                 # 25088

    nc = bacc.Bacc(None, target_bir_lowering=False, num_swdge_queues=NQ)

    def pad512(x):
        return ((x + 511) // 512) * 512

    featT = [nc.dram_tensor(f"featT_{t}", [D_IN[i], pad512(s)], dt.float32,
                            kind="ExternalInput")
             for i, (t, s) in enumerate(zip("abc", (pl.a_s, pl.b_s, pl.c_s)))]
    Waug = [nc.dram_tensor(f"Waug_{t}", [D_IN[i], 65], dt.float32,
                           kind="ExternalInput") for i, t in enumerate("abc")]
    baug = [nc.dram_tensor(f"baug_{t}", [1, 65], dt.float32,
                           kind="ExternalInput") for t in "abc"]
    Wg_aug = nc.dram_tensor("Wg_aug", [65, D], dt.float32, kind="ExternalInput")
    ones_rhs = nc.dram_tensor("ones_rhs", [1, 512], dt.float32, kind="ExternalInput")
    iota2 = nc.dram_tensor("iota2", [P, 2 * P], dt.bfloat16, kind="ExternalInput")
    ident64 = nc.dram_tensor("ident64", [D, D], dt.float32, kind="ExternalInput")
    gidx_d = nc.dram_tensor("gidx", [P, S_tot // 16], dt.int16, kind="ExternalInput")
    dstloc_d = nc.dram_tensor("dstloc", [P, T_tot], dt.float32, kind="ExternalInput")
    wcol_d = nc.dram_tensor("wcol", [P, T_tot], dt.float32, kind="ExternalInput")
    acc_out = nc.dram_tensor("acc_out", [P, OUT_W], dt.float32, kind="ExternalOutput")

    # table rows are 256B (128 bf16); cols 64..127 are never consumed, so
    # they may hold garbage -- only cols :64 are written / read.
    # The slice is split in two halves with separate AllGathers into separate
    # tensors, so half-0 gathers overlap the half-1 collective.
    HALF = pl.HALF
    slice_h = [nc.dram_tensor(f"slice_h{h}", [HALF, 2 * D], dt.bfloat16)
               for h in range(2)]
    table_h = [nc.dram_tensor(f"table_h{h}", [N // 2, 2 * D], dt.bfloat16,
                              addr_space="Shared") for h in range(2)]

    with tile.TileContext(nc) as tc:
        with tc.tile_pool(name="const", bufs=1) as cpool:
            iota_sb = cpool.tile([P, 2 * P], dt.bfloat16)
            nc.sync.dma_start(out=iota_sb[:], in_=iota2[:])
            ones_sb = cpool.tile([1, 512], dt.float32)
            nc.sync.dma_start(out=ones_sb[:], in_=ones_rhs[:])
            wg_sb = cpool.tile([65, D], dt.float32)
            nc.sync.dma_start(out=wg_sb[:], in_=Wg_aug[:])

            # ============== phase 1: support slice =====================
            with (
                tc.tile_pool(name="p1w", bufs=1) as p1w,
                tc.tile_pool(name="p1f", bufs=3) as p1f,
                tc.tile_pool(name="p1s", bufs=3) as p1s,
                tc.tile_pool(name="p1o", bufs=3) as p1o,
                tc.tile_pool(name="psum1", bufs=2, space="PSUM") as psum1,
                tc.tile_pool(name="psum2", bufs=2, space="PSUM") as psum2,
            ):
                for i, (rows, row0) in enumerate(
                    ((pl.a_s, 0), (pl.b_s, pl.a_s), (pl.c_s, pl.a_s + pl.b_s))
                    if do_p1 else ()
                ):
                    din = D_IN[i]
                    nk = din // P
                    waug_sb = [p1w.tile([P, 65], dt.float32, tag=f"waug{i}_{kk}",
                                        name=f"waug{i}_{kk}")
                               for kk in range(nk)]
                    for kk in range(nk):
                        nc.sync.dma_start(out=waug_sb[kk][:],
                                          in_=Waug[i][kk * P:(kk + 1) * P, :])
                    baug_sb = p1w.tile([1, 65], dt.float32, tag=f"baug{i}")
                    nc.sync.dma_start(out=baug_sb[:], in_=baug[i][:])
                    for j in range((rows + 511) // 512):
                        c0 = j * 512
                        fts = []
                        for kk in range(nk):
                            ft = p1f.tile([P, 512], dt.float32, tag="ft")
                            nc.sync.dma_start(
                                out=ft[:],
                                in_=featT[i][kk * P:(kk + 1) * P, c0:c0 + 512])
                            fts.append(ft)
                        encT_ps = psum1.tile([65, 512], dt.float32, tag="encT")
                        for kk in range(nk):
                            nc.tensor.matmul(
                                out=encT_ps[:],
                                lhsT=waug_sb[kk][:],
                                rhs=fts[kk][:],
                                start=(kk == 0), stop=False, skip_group_check=True)
                        nc.tensor.matmul(out=encT_ps[:], lhsT=baug_sb[:],
                                         rhs=ones_sb[:], start=False, stop=True,
                                         skip_group_check=True)
                        encT_sb = p1s.tile([65, 512], dt.float32, tag="encT_sb")
                        nc.scalar.activation(encT_sb[:], encT_ps[:], Act.Relu)
                        sup_ps = psum2.tile([P, 256], dt.float32, tag="sup")
                        # start=True zeroes the whole PSUM bank, so only the
                        # first matmul of the bank sets it.
                        for q in range(4):
                            nc.tensor.matmul(
                                out=sup_ps[:, q * D:(q + 1) * D],
                                lhsT=encT_sb[:, q * P:(q + 1) * P], rhs=wg_sb[:],
                                start=(q == 0), stop=(q == 3),
                                skip_group_check=True)
                        sup_sb = p1o.tile([P, 4, D], dt.bfloat16, tag="sup_sb")
                        nc.vector.tensor_copy(
                            out=sup_sb[:],
                            in_=sup_ps[:].rearrange("p (q d) -> p q d", d=D))
                        for q in range(4):
                            r0 = c0 + q * P
                            v = min(P, rows - r0)
                            if v <= 0:
                                break
                            gr = row0 + r0
                            h, hr = divmod(gr, HALF)
                            nc.sync.dma_start(
                                out=slice_h[h][hr:hr + v, :D],
                                in_=sup_sb[:v, q, :])

            # ============== allgather ==================================
            if do_ag:
                for h in range(2):
                    nc.gpsimd.collective_compute(
                        "AllGather", Alu.bypass,
                        replica_groups=[list(range(NCORES))],
                        ins=[slice_h[h][:]], outs=[table_h[h][:]])

            # ============== phase 2: gather + segment matmul ===========
            with tc.tile_pool(name="acc", bufs=1) as accpool:
                ident_sb = accpool.tile([D, D], dt.float32)
                nc.sync.dma_start(out=ident_sb[:], in_=ident64[:])
                norm_sb = accpool.tile([P, OUT_W], dt.float32, tag="norm")
                if not do_mm or not do_p2:
                    nc.vector.memset(norm_sb[:], 0.0)
                with (
                    tc.tile_pool(name="gpool", bufs=12) as gpool,
                    tc.tile_pool(name="ipool", bufs=4) as ipool,
                    tc.tile_pool(name="opool", bufs=2) as opool,
                    tc.tile_pool(name="mpool", bufs=16) as mpool,
                    tc.tile_pool(name="flpool", bufs=2) as flpool,
                    tc.tile_pool(name="psacc", bufs=1, space="PSUM") as psacc,
                    tc.tile_pool(name="tpool", bufs=2, space="PSUM") as tpool,
                ):
                    gctr = 0
                    for g in range(NG if do_p2 else 0):
                        spg = pl.pairs_in_group[g]
                        nbanks = (spg + 1) // 2
                        Tg = int(T[g].sum())
                        t0_g = int(T[:g].sum())
                        dl_sb = opool.tile([P, Tg], dt.float32, tag="dl")
                        nc.sync.dma_start(out=dl_sb[:], in_=dstloc_d[:, t0_g:t0_g + Tg])
                        wc_sb = opool.tile([P, Tg], dt.float32, tag="wc")
                        nc.sync.dma_start(out=wc_sb[:], in_=wcol_d[:, t0_g:t0_g + Tg])

                        banks = [psacc.tile([P, 512], dt.float32, tag=f"bank{b}",
                                           name=f"bank_g{g}_{b}")
                                 for b in range(nbanks)] if do_mm else []
                        bank_first = [True] * nbanks
                        # execution-order last tile per bank, for stop flags
                        last_of_bank = {}
                        for sw_ in range(NSW):
                            for sp_ in range(spg):
                                nt_ = int(T[g, sw_, sp_])
                                if nt_ > 0:
                                    last_of_bank[sp_ // 2] = (sw_, sp_, nt_ - 1)
                        tcol = 0
                        for sw in range(NSW):
                            Tsw = int(T[g, sw].sum())
                            s0 = int(pl.cell_base[g, sw, 0])
                            nslots = Tsw * P
                            it = ipool.tile([P, nslots // 16], dt.int16, tag="idx")
                            nc.sync.dma_start(
                                out=it[:], in_=gidx_d[:, s0 // 16:(s0 + nslots) // 16])
                            chunks = []
                            off = 0
                            while off < nslots:
                                ln = min(GATHER_CHUNK, nslots - off)
                                gt = gpool.tile([P, GATHER_CHUNK // P, 2 * D],
                                                dt.bfloat16, tag="gat", name="gt") \
                                    if do_gather else None
                                if do_gather:
                                    nc.gpsimd.dma_gather(
                                        out_ap=gt[:, :ln // P, :],
                                        in_ap=table_h[sw // 4][
                                            (sw % 4) * pl.SW:
                                            (sw % 4 + 1) * pl.SW, :],
                                        idxs_ap=it[:, off // 16:(off + ln) // 16],
                                        num_idxs=ln, num_idxs_reg=ln, elem_size=2 * D,
                                        queue_num=gctr % NQ)
                                gctr += 1
                                chunks.append(gt)
                                off += ln
                            tloc = 0
                            for sp in range(spg):
                                nt = int(T[g, sw, sp])
                                bank, half = sp // 2, sp % 2
                                for t in range(nt):
                                    slot0 = (tloc + t) * P
                                    gt = chunks[slot0 // GATHER_CHUNK]
                                    kk = (slot0 % GATHER_CHUNK) // P
                                    col = tcol + tloc + t
                                    m = mpool.tile([P, 2 * P], dt.bfloat16, tag="m")
                                    if do_mb:
                                        nc.vector.tensor_scalar(
                                            out=m[:], in0=iota_sb[:],
                                            scalar1=dl_sb[:, col:col + 1],
                                            scalar2=wc_sb[:, col:col + 1],
                                            op0=Alu.is_equal, op1=Alu.mult)
                                    if do_mm:
                                        nc.tensor.matmul(
                                            out=banks[bank][:D, half * 256:
                                                            half * 256 + 256],
                                            lhsT=gt[:, kk, :D],
                                            rhs=m[:],
                                            start=bank_first[bank],
                                            stop=(last_of_bank[bank] == (sw, sp, t)),
                                            skip_group_check=True)
                                        bank_first[bank] = False
                                tloc += nt
                            tcol += tloc
                        # flush psum -> sbuf staging, transpose back to
                        # row-major, stream into norm_sb (overlaps next group)
                        if do_mm:
                            fl = flpool.tile([D, spg * 256], dt.float32, tag="fl")
                            for b in range(nbanks):
                                pcols = min(2, spg - 2 * b) * 256
                                if b in last_of_bank:
                                    nc.vector.tensor_copy(
                                        out=fl[:, b * 512:b * 512 + pcols],
                                        in_=banks[b][:D, :pcols])
                                else:
                                    nc.vector.memset(
                                        fl[:, b * 512:b * 512 + pcols], 0.0)
                            nblk_g = spg * 2
                            jb = 0
                            while jb < nblk_g:
                                w = min(8, nblk_g - jb)
                                tpb = tpool.tile([P, 512], dt.float32, tag="tp")
                                for k in range(w):
                                    nc.tensor.matmul(
                                        out=tpb[:, k * D:(k + 1) * D],
                                        lhsT=fl[:, (jb + k) * P:(jb + k + 1) * P],
                                        rhs=ident_sb[:], is_transpose=True,
                                        start=(k == 0), stop=(k == w - 1),
                                        skip_group_check=True)
                                j0 = g * PAIRS_PER_GROUP * 2 + jb
                                nc.scalar.activation(
                                    norm_sb[:, j0 * D:(j0 + w) * D],
                                    tpb[:, :w * D], Act.Copy)
                                jb += w

                # ============== phase 3: normalize =====================
                with tc.tile_pool(name="npool", bufs=1) as npool:
                    sq = npool.tile([P, OUT_W], dt.float32, tag="sq")
                    nc.vector.tensor_tensor(out=sq[:], in0=norm_sb[:],
                                            in1=norm_sb[:], op=Alu.mult)
                    ss = npool.tile([P, NBLK], dt.float32, tag="ss")
                    nc.vector.tensor_reduce(
                        out=ss[:],
                        in_=sq[:].rearrange("p (b d) -> p b d", d=D),
                        axis=mybir.AxisListType.X, op=Alu.add)
                    nrm = npool.tile([P, NBLK], dt.float32, tag="nrm")
                    nc.scalar.activation(nrm[:], ss[:], Act.Sqrt)
                    nc.vector.tensor_scalar_max(nrm[:], nrm[:], 1e-12)
                    rec = npool.tile([P, NBLK], dt.float32, tag="rec")
                    nc.vector.reciprocal(rec[:], nrm[:])
                    rap = rec[:]
                    rec_b = AP(rap.tensor, rap.offset, list(rap.ap) + [[0, D]])
                    nc.vector.tensor_tensor(
                        out=norm_sb[:].rearrange("p (b d) -> p b d", d=D),
                        in0=norm_sb[:].rearrange("p (b d) -> p b d", d=D),
                        in1=rec_b, op=Alu.mult)
                    nc.sync.dma_start(out=acc_out[:], in_=norm_sb[:])

    nc.compile()
    return nc


def _in_maps(pl, inputs):
    feats = [np.asarray(inputs["feat_a"], np.float32),
             np.asarray(inputs["feat_b"], np.float32),
             np.asarray(inputs["feat_c"], np.float32)]
    Ws = [np.asarray(inputs["W_a"], np.float32),
          np.asarray(inputs["W_b"], np.float32),
          np.asarray(inputs["W_c"], np.float32)]
    bs = [np.asarray(inputs["b_a"], np.float32),
          np.asarray(inputs["b_b"], np.float32),
          np.asarray(inputs["b_c"], np.float32)]
    gcn_W = np.asarray(inputs["gcn_W"], np.float32)
    gcn_b = np.asarray(inputs["gcn_b"], np.float32)

    Waug = [np.ascontiguousarray(
        np.concatenate([W, np.zeros((W.shape[0], 1), np.float32)], 1))
        for W in Ws]
    baug = [np.ascontiguousarray(
        np.concatenate([b, np.ones(1, np.float32)])[None, :]) for b in bs]
    Wg_aug = np.ascontiguousarray(np.concatenate([gcn_W, gcn_b[None, :]], 0))
    ones = np.ones((1, 512), np.float32)
    iota2 = np.ascontiguousarray(
        _bf16(np.tile(np.arange(2 * P, dtype=np.float32)[None, :], (P, 1))))
    ident = np.eye(D, dtype=np.float32)

    sizes = [pl.a_s, pl.b_s, pl.c_s]
    maps = []
    for c in range(NCORES):
        m = {}
        for t, nm in enumerate("abc"):
            cnt = sizes[t]
            rows = feats[t][c * cnt:(c + 1) * cnt]
            padcols = ((cnt + 511) // 512) * 512
            ft = np.zeros((D_IN[t], padcols), np.float32)
            ft[:, :cnt] = rows.T
            m[f"featT_{nm}"] = ft
            m[f"Waug_{nm}"] = Waug[t]
            m[f"baug_{nm}"] = baug[t]
        m["Wg_aug"] = Wg_aug
        m["ones_rhs"] = ones
        m["iota2"] = iota2
        m["ident64"] = ident
        m["gidx"] = pl.gidx_wrapped[c]
        m["dstloc"] = pl.dstloc[c]
        m["wcol"] = pl.wcol[c]
        maps.append(m)
    return maps


def _unshard(pl, results):
    NBLK = (pl.NPAIR * PAIR) // P
    out = np.zeros((pl.N, D), np.float32)
    for c in range(NCORES):
        acc = results[c]["acc_out"].reshape(P, NBLK, D)
        for j in range(NBLK):
            d0 = j * P
            v = min(P, pl.SLICE - d0)
            if v <= 0:
                break
            rows = pl.table_to_node[c * pl.SLICE + d0 + np.arange(v)]
            out[rows] = acc[:v, j, :]
    return out


def kernel(**inputs):
    from concourse.bass_utils import run_bass_kernel_spmd

    edge_src = np.asarray(inputs["edge_src"]).astype(np.int64)
    edge_dst = np.asarray(inputs["edge_dst"]).astype(np.int64)
    edge_weight = np.asarray(inputs["edge_weight"], dtype=np.float32)

    pl = _make_plan(edge_src, edge_dst, edge_weight)
    nc = _build_nc(pl)
    maps = _in_maps(pl, inputs)
    res = run_bass_kernel_spmd(nc, maps, core_ids=list(range(NCORES)))
    return _unshard(pl, [res.results[c] for c in range(NCORES)])


# revision 30
# speedup vs baseline: 1.4125x; 1.4125x over previous
"""Trainium2 Bass kernel for NSHE-style GNN message passing.

  enc = relu(concat(feat_a@W_a+b_a, feat_b@W_b+b_b, feat_c@W_c+b_c))
  support = enc @ gcn_W + gcn_b
  msg = support[edge_src] * edge_weight[:, None]
  com = segment_sum(msg, edge_dst, N);  out = l2_normalize(com, axis=1)

Distribution (8 NeuronCores, one shared SPMD NEFF):
  - nodes are permuted into 8 balanced per-core slices (each slice mixes the
    three feature types so per-core phase-1 work is equal); core k computes
    `support` rows for its slice (bf16, rows padded to 256B for the gather
    granule). The slice is split in two halves with two pipelined AllGathers
    into separate [N/2, 128] tables laid out [half, core, half-slice]: the
    first collective fires as soon as the type-a rows (exactly half 0) are
    done and overlaps the rest of phase 1; gathers over half-0 windows
    overlap the second collective.
  - edges are partitioned by destination slice; each core's edge stream is
    sorted by (psum-group, src-window, pair-window, src) and padded so tile
    counts are identical on every core (one program serves all cores).
  - support[src] rows are fetched with dma_gather (SWDGE, int16 indices into
    25000-row windows, <=1024 idxs/instruction, spread over 4 SWDGE queues).
  - segment-sum runs on the tensor engine: per 128-edge tile a selection
    matrix M[e, j] = (j == dst_rel[e]) * w_e (bf16) is built with one DVE
    tensor_scalar; ONE matmul per tile computes psum[f, dst] += gt^T @ M
    with the gathered rows as the 64-col stationary operand and M as the
    256-col moving operand. Each PSUM bank holds two 256-dst pairs
    ([64, 512]); 12 pairs per group, two banks serve the PE transposes
    that stream each finished group back to row-major [dst, 64].
  - the [64, dst] accumulator is PE-transposed back to row-major [dst, 64],
    l2-normalized in one batched pass; the host undoes the permutation.
"""

import numpy as np

N_A, N_B, N_C = 100000, 60000, 40000
D = 64
D_IN = (512, 256, 128)
NCORES = 8

P = 128                  # partitions / edge-tile size
PAIR = 2 * P             # dst span covered by one tile's matmul
PAIRS_PER_GROUP = 12     # 6 psum banks x 2 pairs; 2 banks for transposes
GATHER_CHUNK = 1024      # idxs per dma_gather (SWDGE ring limit)
NQ = 4                   # SWDGE queues


class _Plan:
    pass


def _bf16(x):
    import ml_dtypes
    return np.asarray(x).astype(ml_dtypes.bfloat16)


def _make_plan(edge_src, edge_dst, edge_weight):
    """Host-side sharding: node permutation, uniform per-core edge schedule,
    operand arrays. Index manipulation only -- all float math runs on device
    (edge weights are moved, never combined, here)."""
    pl = _Plan()
    N = N_A + N_B + N_C
    SLICE = N // NCORES
    a_s, b_s, c_s = N_A // NCORES, N_B // NCORES, N_C // NCORES

    node_to_table = np.empty(N, dtype=np.int64)
    karr = np.arange(NCORES)
    for cnt, node0, off in ((a_s, 0, 0), (b_s, N_A, a_s), (c_s, N_A + N_B, a_s + b_s)):
        idx = node0 + (karr[:, None] * cnt + np.arange(cnt)[None, :])
        rows = SLICE * karr[:, None] + off + np.arange(cnt)[None, :]
        node_to_table[idx.ravel()] = rows.ravel()
    table_to_node = np.empty(N, dtype=np.int64)
    table_to_node[node_to_table] = np.arange(N)
    pl.N, pl.SLICE = N, SLICE
    pl.a_s, pl.b_s, pl.c_s = a_s, b_s, c_s
    pl.node_to_table, pl.table_to_node = node_to_table, table_to_node

    # src-side rows follow the split-AllGather layout: half h of every
    # core's slice is gathered into table_h[h] with rows [core, half-slice],
    # so src windows of 12500 rows each map to one core's half.
    HALF = SLICE // 2
    c_of = node_to_table // SLICE
    r_of = node_to_table % SLICE
    ag_row = (r_of // HALF) * (NCORES * HALF) + c_of * HALF + (r_of % HALF)
    pl.HALF = HALF
    src_t = ag_row[edge_src]
    dst_t = node_to_table[edge_dst]
    w = edge_weight

    core = dst_t // SLICE
    dst_local = dst_t - core * SLICE
    pair = dst_local // PAIR
    NPAIR = (SLICE + PAIR - 1) // PAIR
    NG = (NPAIR + PAIRS_PER_GROUP - 1) // PAIRS_PER_GROUP
    pl.NPAIR, pl.NG = NPAIR, NG
    pl.pairs_in_group = [min(PAIRS_PER_GROUP, NPAIR - g * PAIRS_PER_GROUP)
                         for g in range(NG)]
    g_of = pair // PAIRS_PER_GROUP
    sp_of = pair % PAIRS_PER_GROUP
    SW = SLICE                   # src window: 25000 contiguous table rows
    assert SW <= 32767           # = two cores' half-slices (int16-safe)
    NSW = N // SW
    pl.NSW, pl.SW = NSW, SW
    sw = src_t // SW

    cell = ((core * NG + g_of) * NSW + sw) * PAIRS_PER_GROUP + sp_of
    counts = np.bincount(cell, minlength=NCORES * NG * NSW * PAIRS_PER_GROUP)
    counts = counts.reshape(NCORES, NG, NSW, PAIRS_PER_GROUP)
    T = np.maximum(1, -(-counts.max(axis=0) // P))          # [NG, NSW, SPG]
    for g in range(NG):
        T[g, :, pl.pairs_in_group[g]:] = 0
    pl.T = T
    T_tot = int(T.sum())
    S_tot = T_tot * P
    pl.T_tot, pl.S_tot = T_tot, S_tot

    cell_sizes = (T * P).ravel()
    cb = np.zeros(len(cell_sizes) + 1, dtype=np.int64)
    np.cumsum(cell_sizes, out=cb[1:])
    pl.cell_base = cb[:-1].reshape(NG, NSW, PAIRS_PER_GROUP)

    gidx = np.zeros((NCORES, S_tot), dtype=np.int16)
    dstrel = np.full((NCORES, S_tot), -999.0, dtype=np.float32)
    wstream = np.zeros((NCORES, S_tot), dtype=np.float32)

    order = np.lexsort((src_t, sp_of, sw, g_of, core))
    srt_core = core[order]
    srt_cic = (g_of[order] * NSW + sw[order]) * PAIRS_PER_GROUP + sp_of[order]
    srt_src_rel = (src_t[order] - sw[order] * SW).astype(np.int16)
    srt_dst_rel = (dst_local[order] - pair[order] * PAIR).astype(np.float32)
    srt_w = w[order].astype(np.float32)

    flat_base = pl.cell_base.ravel()
    ncell_pc = NG * NSW * PAIRS_PER_GROUP
    for c in range(NCORES):
        m = srt_core == c
        cic = srt_cic[m]
        oc = np.bincount(cic, minlength=ncell_pc)
        within = (np.arange(len(cic)) - np.repeat(np.concatenate([[0], np.cumsum(oc)[:-1]]), oc))
        pos = flat_base[cic] + within
        gidx[c, pos] = srt_src_rel[m]
        dstrel[c, pos] = srt_dst_rel[m]
        wstream[c, pos] = srt_w[m]

    pl.dstloc = dstrel.reshape(NCORES, T_tot, P).transpose(0, 2, 1).copy()
    pl.wcol = wstream.reshape(NCORES, T_tot, P).transpose(0, 2, 1).copy()
    wrapped = gidx.reshape(NCORES, S_tot // 16, 16).transpose(0, 2, 1)
    pl.gidx_wrapped = np.ascontiguousarray(np.tile(wrapped, (1, 8, 1)))
    return pl


def _build_nc(pl, variant="full"):
    # variant: ablation knob for benchmarking ("full" for real use).
    # cumulative ladder: each step also drops downstream consumers so no
    # tile is read-but-never-written (the allocator rejects that).
    #   nomm: drop segment matmuls + flush + transpose/normalize content
    #   nomb: also drop M-build     nog: also drop gathers
    #   p1ag: skip whole phase-2 loop    nop1 / noag: drop that phase
    import concourse.bacc as bacc
    import concourse.mybir as mybir
    import concourse.tile as tile
    from concourse.bass import AP

    do_p1 = variant != "nop1"
    do_ag = variant != "noag"
    do_mm = variant not in ("nomm", "nomb", "nog")
    do_mb = variant not in ("nomb", "nog")
    do_gather = variant != "nog"
    do_p2 = variant != "p1ag"

    dt = mybir.dt
    Alu = mybir.AluOpType
    Act = mybir.ActivationFunctionType
    NG, NSW, SLICE, T = pl.NG, pl.NSW, pl.SLICE, pl.T
    T_tot, S_tot, N = pl.T_tot, pl.S_tot, pl.N
    NBLK = (pl.NPAIR * PAIR) // P           # 196 transposed output blocks
    OUT_W = NBLK * D                        # 12544
    ACC_W = pl.NPAIR * PAIR                 # 25088

    nc = bacc.Bacc(None, target_bir_lowering=False, num_swdge_queues=NQ)

    def pad512(x):
        return ((x + 511) // 512) * 512

    featT = [nc.dram_tensor(f"featT_{t}", [D_IN[i], pad512(s)], dt.float32,
                            kind="ExternalInput")
             for i, (t, s) in enumerate(zip("abc", (pl.a_s, pl.b_s, pl.c_s)))]
    Waug = [nc.dram_tensor(f"Waug_{t}", [D_IN[i], 65], dt.float32,
                           kind="ExternalInput") for i, t in enumerate("abc")]
    baug = [nc.dram_tensor(f"baug_{t}", [1, 65], dt.float32,
                           kind="ExternalInput") for t in "abc"]
    Wg_aug = nc.dram_tensor("Wg_aug", [65, D], dt.float32, kind="ExternalInput")
    ones_rhs = nc.dram_tensor("ones_rhs", [1, 512], dt.float32, kind="ExternalInput")
    iota2 = nc.dram_tensor("iota2", [P, 2 * P], dt.bfloat16, kind="ExternalInput")
    ident64 = nc.dram_tensor("ident64", [D, D], dt.float32, kind="ExternalInput")
    gidx_d = nc.dram_tensor("gidx", [P, S_tot // 16], dt.int16, kind="ExternalInput")
    dstloc_d = nc.dram_tensor("dstloc", [P, T_tot], dt.float32, kind="ExternalInput")
    wcol_d = nc.dram_tensor("wcol", [P, T_tot], dt.float32, kind="ExternalInput")
    acc_out = nc.dram_tensor("acc_out", [P, OUT_W], dt.float32, kind="ExternalOutput")

    # table rows are 256B (128 bf16); cols 64..127 are never consumed, so
    # they may hold garbage -- only cols :64 are written / read.
    # The slice is split in two halves with separate AllGathers into separate
    # tensors, so half-0 gathers overlap the half-1 collective.
    HALF = pl.HALF
    slice_h = [nc.dram_tensor(f"slice_h{h}", [HALF, 2 * D], dt.bfloat16)
               for h in range(2)]
    table_h = [nc.dram_tensor(f"table_h{h}", [N // 2, 2 * D], dt.bfloat16,
                              addr_space="Shared") for h in range(2)]

    with tile.TileContext(nc) as tc:
        with tc.tile_pool(name="const", bufs=1) as cpool:
            iota_sb = cpool.tile([P, 2 * P], dt.bfloat16)
            nc.sync.dma_start(out=iota_sb[:], in_=iota2[:])
            ones_sb = cpool.tile([1, 512], dt.float32)
            nc.sync.dma_start(out=ones_sb[:], in_=ones_rhs[:])
            wg_sb = cpool.tile([65, D], dt.float32)
            nc.sync.dma_start(out=wg_sb[:], in_=Wg_aug[:])

            # ============== phase 1: support slice =====================
            with (
                tc.tile_pool(name="p1w", bufs=1) as p1w,
                tc.tile_pool(name="p1f", bufs=3) as p1f,
                tc.tile_pool(name="p1s", bufs=3) as p1s,
                tc.tile_pool(name="p1o", bufs=3) as p1o,
                tc.tile_pool(name="psum1", bufs=2, space="PSUM") as psum1,
                tc.tile_pool(name="psum2", bufs=2, space="PSUM") as psum2,
            ):
                for i, (rows, row0) in enumerate(
                    ((pl.a_s, 0), (pl.b_s, pl.a_s), (pl.c_s, pl.a_s + pl.b_s))
                    if do_p1 else ()
                ):
                    din = D_IN[i]
                    nk = din // P
                    waug_sb = [p1w.tile([P, 65], dt.float32, tag=f"waug{i}_{kk}",
                                        name=f"waug{i}_{kk}")
                               for kk in range(nk)]
                    for kk in range(nk):
                        nc.sync.dma_start(out=waug_sb[kk][:],
                                          in_=Waug[i][kk * P:(kk + 1) * P, :])
                    baug_sb = p1w.tile([1, 65], dt.float32, tag=f"baug{i}")
                    nc.sync.dma_start(out=baug_sb[:], in_=baug[i][:])
                    for j in range((rows + 511) // 512):
                        c0 = j * 512
                        fts = []
                        for kk in range(nk):
                            ft = p1f.tile([P, 512], dt.float32, tag="ft")
                            nc.sync.dma_start(
                                out=ft[:],
                                in_=featT[i][kk * P:(kk + 1) * P, c0:c0 + 512])
                            fts.append(ft)
                        encT_ps = psum1.tile([65, 512], dt.float32, tag="encT")
                        for kk in range(nk):
                            nc.tensor.matmul(
                                out=encT_ps[:],
                                lhsT=waug_sb[kk][:],
                                rhs=fts[kk][:],
                                start=(kk == 0), stop=False, skip_group_check=True)
                        nc.tensor.matmul(out=encT_ps[:], lhsT=baug_sb[:],
                                         rhs=ones_sb[:], start=False, stop=True,
                                         skip_group_check=True)
                        encT_sb = p1s.tile([65, 512], dt.float32, tag="encT_sb")
                        nc.scalar.activation(encT_sb[:], encT_ps[:], Act.Relu)
                        sup_ps = psum2.tile([P, 256], dt.float32, tag="sup")
                        # start=True zeroes the whole PSUM bank, so only the
                        # first matmul of the bank sets it.
                        for q in range(4):
                            nc.tensor.matmul(
                                out=sup_ps[:, q * D:(q + 1) * D],
                                lhsT=encT_sb[:, q * P:(q + 1) * P], rhs=wg_sb[:],
                                start=(q == 0), stop=(q == 3),
                                skip_group_check=True)
                        sup_sb = p1o.tile([P, 4, D], dt.bfloat16, tag="sup_sb")
                        nc.vector.tensor_copy(
                            out=sup_sb[:],
                            in_=sup_ps[:].rearrange("p (q d) -> p q d", d=D))
                        for q in range(4):
                            r0 = c0 + q * P
                            v = min(P, rows - r0)
                            if v <= 0:
                                break
                            gr = row0 + r0
                            h, hr = divmod(gr, HALF)
                            nc.sync.dma_start(
                                out=slice_h[h][hr:hr + v, :D],
                                in_=sup_sb[:v, q, :])

            # ============== allgather ==================================
            if do_ag:
                for h in range(2):
                    nc.gpsimd.collective_compute(
                        "AllGather", Alu.bypass,
                        replica_groups=[list(range(NCORES))],
                        ins=[slice_h[h][:]], outs=[table_h[h][:]])

            # ============== phase 2: gather + segment matmul ===========
            with tc.tile_pool(name="acc", bufs=1) as accpool:
                ident_sb = accpool.tile([D, D], dt.float32)
                nc.sync.dma_start(out=ident_sb[:], in_=ident64[:])
                norm_sb = accpool.tile([P, OUT_W], dt.float32, tag="norm")
                if not do_mm or not do_p2:
                    nc.vector.memset(norm_sb[:], 0.0)
                with (
                    tc.tile_pool(name="gpool", bufs=12) as gpool,
                    tc.tile_pool(name="ipool", bufs=4) as ipool,
                    tc.tile_pool(name="opool", bufs=2) as opool,
                    tc.tile_pool(name="mpool", bufs=16) as mpool,
                    tc.tile_pool(name="flpool", bufs=2) as flpool,
                    tc.tile_pool(name="psacc", bufs=1, space="PSUM") as psacc,
                    tc.tile_pool(name="tpool", bufs=2, space="PSUM") as tpool,
                ):
                    gctr = 0
                    for g in range(NG if do_p2 else 0):
                        spg = pl.pairs_in_group[g]
                        nbanks = (spg + 1) // 2
                        Tg = int(T[g].sum())
                        t0_g = int(T[:g].sum())
                        dl_sb = opool.tile([P, Tg], dt.float32, tag="dl")
                        nc.sync.dma_start(out=dl_sb[:], in_=dstloc_d[:, t0_g:t0_g + Tg])
                        wc_sb = opool.tile([P, Tg], dt.float32, tag="wc")
                        nc.sync.dma_start(out=wc_sb[:], in_=wcol_d[:, t0_g:t0_g + Tg])

                        banks = [psacc.tile([P, 512], dt.float32, tag=f"bank{b}",
                                           name=f"bank_g{g}_{b}")
                                 for b in range(nbanks)] if do_mm else []
                        bank_first = [True] * nbanks
                        # execution-order last tile per bank, for stop flags
                        last_of_bank = {}
                        for sw_ in range(NSW):
                            for sp_ in range(spg):
                                nt_ = int(T[g, sw_, sp_])
                                if nt_ > 0:
                                    last_of_bank[sp_ // 2] = (sw_, sp_, nt_ - 1)
                        tcol = 0
                        for sw in range(NSW):
                            Tsw = int(T[g, sw].sum())
                            s0 = int(pl.cell_base[g, sw, 0])
                            nslots = Tsw * P
                            it = ipool.tile([P, nslots // 16], dt.int16, tag="idx")
                            nc.sync.dma_start(
                                out=it[:], in_=gidx_d[:, s0 // 16:(s0 + nslots) // 16])
                            chunks = []
                            off = 0
                            while off < nslots:
                                ln = min(GATHER_CHUNK, nslots - off)
                                gt = gpool.tile([P, GATHER_CHUNK // P, 2 * D],
                                                dt.bfloat16, tag="gat", name="gt") \
                                    if do_gather else None
                                if do_gather:
                                    nc.gpsimd.dma_gather(
                                        out_ap=gt[:, :ln // P, :],
                                        in_ap=table_h[sw // 4][
                                            (sw % 4) * pl.SW:
                                            (sw % 4 + 1) * pl.SW, :],
                                        idxs_ap=it[:, off // 16:(off + ln) // 16],
                                        num_idxs=ln, num_idxs_reg=ln, elem_size=2 * D,
                                        queue_num=gctr % NQ)
                                gctr += 1
                                chunks.append(gt)
                                off += ln
                            tloc = 0
                            for sp in range(spg):
                                nt = int(T[g, sw, sp])
                                bank, half = sp // 2, sp % 2
                                for t in range(nt):
                                    slot0 = (tloc + t) * P
                                    gt = chunks[slot0 // GATHER_CHUNK]
                                    kk = (slot0 % GATHER_CHUNK) // P
                                    col = tcol + tloc + t
                                    m = mpool.tile([P, 2 * P], dt.bfloat16, tag="m")
                                    if do_mb:
                                        nc.vector.tensor_scalar(
                                            out=m[:], in0=iota_sb[:],
                                            scalar1=dl_sb[:, col:col + 1],
                                            scalar2=wc_sb[:, col:col + 1],
                                            op0=Alu.is_equal, op1=Alu.mult)
                                    if do_mm:
                                        nc.tensor.matmul(
                                            out=banks[bank][:D, half * 256:
                                                            half * 256 + 256],
                                            lhsT=gt[:, kk, :D],
                                            rhs=m[:],
                                            start=bank_first[bank],
                                            stop=(last_of_bank[bank] == (sw, sp, t)),
                                            skip_group_check=True)
                                        bank_first[bank] = False
                                tloc += nt
                            tcol += tloc
                        # flush psum -> sbuf staging, transpose back to
                        # row-major, stream into norm_sb (overlaps next group)
                        if do_mm:
                            fl = flpool.tile([D, spg * 256], dt.float32, tag="fl")
                            for b in range(nbanks):
                                pcols = min(2, spg - 2 * b) * 256
                                if b in last_of_bank:
                                    nc.vector.tensor_copy(
                                        out=fl[:, b * 512:b * 512 + pcols],
                                        in_=banks[b][:D, :pcols])
                                else:
                                    nc.vector.memset(
                                        fl[:, b * 512:b * 512 + pcols], 0.0)
                            nblk_g = spg * 2
                            jb = 0
                            while jb < nblk_g:
                                w = min(8, nblk_g - jb)
                                tpb = tpool.tile([P, 512], dt.float32, tag="tp")
                                for k in range(w):
                                    nc.tensor.matmul(
                                        out=tpb[:, k * D:(k + 1) * D],
                                        lhsT=fl[:, (jb + k) * P:(jb + k + 1) * P],
                                        rhs=ident_sb[:], is_transpose=True,
                                        start=(k == 0), stop=(k == w - 1),
                                        skip_group_check=True)
                                j0 = g * PAIRS_PER_GROUP * 2 + jb
                                nc.scalar.activation(
                                    norm_sb[:, j0 * D:(j0 + w) * D],
                                    tpb[:, :w * D], Act.Copy)
                                jb += w

                # ============== phase 3: normalize =====================
                with tc.tile_pool(name="npool", bufs=1) as npool:
                    sq = npool.tile([P, OUT_W], dt.float32, tag="sq")
                    nc.vector.tensor_tensor(out=sq[:], in0=norm_sb[:],
                                            in1=norm_sb[:], op=Alu.mult)
                    ss = npool.tile([P, NBLK], dt.float32, tag="ss")
                    nc.vector.tensor_reduce(
                        out=ss[:],
                        in_=sq[:].rearrange("p (b d) -> p b d", d=D),
                        axis=mybir.AxisListType.X, op=Alu.add)
                    nrm = npool.tile([P, NBLK], dt.float32, tag="nrm")
                    nc.scalar.activation(nrm[:], ss[:], Act.Sqrt)
                    nc.vector.tensor_scalar_max(nrm[:], nrm[:], 1e-12)
                    rec = npool.tile([P, NBLK], dt.float32, tag="rec")
                    nc.vector.reciprocal(rec[:], nrm[:])
                    rap = rec[:]
                    rec_b = AP(rap.tensor, rap.offset, list(rap.ap) + [[0, D]])
                    nc.vector.tensor_tensor(
                        out=norm_sb[:].rearrange("p (b d) -> p b d", d=D),
                        in0=norm_sb[:].rearrange("p (b d) -> p b d", d=D),
                        in1=rec_b, op=Alu.mult)
                    nc.sync.dma_start(out=acc_out[:], in_=norm_sb[:])

    nc.compile()
    return nc


def _in_maps(pl, inputs):
    feats = [np.asarray(inputs["feat_a"], np.float32),
             np.asarray(inputs["feat_b"], np.float32),
             np.asarray(inputs["feat_c"], np.float32)]
    Ws = [np.asarray(inputs["W_a"], np.float32),
          np.asarray(inputs["W_b"], np.float32),
          np.asarray(inputs["W_c"], np.float32)]
    bs = [np.asarray(inputs["b_a"], np.float32),
          np.asarray(inputs["b_b"], np.float32),
          np.asarray(inputs["b_c"], np.float32)]
    gcn_W = np.asarray(inputs["gcn_W"], np.float32)
    gcn_b = np.asarray(inputs["gcn_b"], np.float32)

    Waug = [np.ascontiguousarray(
        np.concatenate([W, np.zeros((W.shape[0], 1), np.float32)], 1))
        for W in Ws]
    baug = [np.ascontiguousarray(
        np.concatenate([b, np.ones(1, np.float32)])[None, :]) for b in bs]
    Wg_aug = np.ascontiguousarray(np.concatenate([gcn_W, gcn_b[None, :]], 0))
    ones = np.ones((1, 512), np.float32)
    iota2 = np.ascontiguousarray(
        _bf16(np.tile(np.arange(2 * P, dtype=np.float32)[None, :], (P, 1))))
    ident = np.eye(D, dtype=np.float32)

    sizes = [pl.a_s, pl.b_s, pl.c_s]
    maps = []
    for c in range(NCORES):
        m = {}
        for t, nm in enumerate("abc"):
            cnt = sizes[t]
            rows = feats[t][c * cnt:(c + 1) * cnt]
            padcols = ((cnt + 511) // 512) * 512
            ft = np.zeros((D_IN[t], padcols), np.float32)
            ft[:, :cnt] = rows.T
            m[f"featT_{nm}"] = ft
            m[f"Waug_{nm}"] = Waug[t]
            m[f"baug_{nm}"] = baug[t]
        m["Wg_aug"] = Wg_aug
        m["ones_rhs"] = ones
        m["iota2"] = iota2
        m["ident64"] = ident
        m["gidx"] = pl.gidx_wrapped[c]
        m["dstloc"] = pl.dstloc[c]
        m["wcol"] = pl.wcol[c]
        maps.append(m)
    return maps


def _unshard(pl, results):
    NBLK = (pl.NPAIR * PAIR) // P
    out = np.zeros((pl.N, D), np.float32)
    for c in range(NCORES):
        acc = results[c]["acc_out"].reshape(P, NBLK, D)
        for j in range(NBLK):
            d0 = j * P
            v = min(P, pl.SLICE - d0)
            if v <= 0:
                break
            rows = pl.table_to_node[c * pl.SLICE + d0 + np.arange(v)]
            out[rows] = acc[:v, j, :]
    return out


def kernel(**inputs):
    from concourse.bass_utils import run_bass_kernel_spmd

    edge_src = np.asarray(inputs["edge_src"]).astype(np.int64)
    edge_dst = np.asarray(inputs["edge_dst"]).astype(np.int64)
    edge_weight = np.asarray(inputs["edge_weight"], dtype=np.float32)

    pl = _make_plan(edge_src, edge_dst, edge_weight)
    nc = _build_nc(pl)
    maps = _in_maps(pl, inputs)
    res = run_bass_kernel_spmd(nc, maps, core_ids=list(range(NCORES)))
    return _unshard(pl, [res.results[c] for c in range(NCORES)])
